# revision 8
# baseline (speedup 1.0000x reference)
"""BitNet transformer block on 8 Trainium2 NeuronCores — v3.

Fully column-parallel dataflow with host-quantized ternary weights (bf16,
exact) and half-split (= per-batch) collectives for compute/collective
overlap:

  - every activation matrix crossing cores travels TRANSPOSED [feature, token]
    through bf16 AllGathers whose outputs are directly matmul-consumable with
    contiguous per-feature-block loads;
  - each core quantizes + transposes only its own feature shard (one batched
    DMA_TRANSPOSE per 128-token tile, Sync queue only — concurrent XBAR
    transposes on two queues corrupt data);
  - rmsnorm statistics and quantization scales are tiny [tokens,1] AllReduces;
  - no ReduceScatter anywhere; the final output is column-sharded [R, 256]
    per core and the host concatenates.

All BitNet matmuls are exact integer arithmetic (int8-grid activations x
ternary weights in bf16, fp32 PSUM accumulate), matching the fp32 reference.
"""

import numpy as np
import ml_dtypes

import concourse.bacc as bacc
import concourse.mybir as mybir
import concourse.tile as tile
from concourse.bass_utils import run_bass_kernel_spmd

F32 = mybir.dt.float32
BF16 = mybir.dt.bfloat16
AF = mybir.ActivationFunctionType
ALU = mybir.AluOpType
AX = mybir.AxisListType

NCORES = 8
B, S, D, H, MLP = 2, 1024, 2048, 16, 8192
HD = 128
R = B * S                 # 2048 token rows total
OQ = D // NCORES          # 256 d-cols per core (2 heads)
OM = MLP // NCORES        # 1024 mlp cols per core
P = 128
KT = D // P               # 16 feature chunks
RT = R // P               # 16 token tiles
ST = S // P               # 8 token tiles per half (= batch)
KM = OM // P              # 8 local mlp-dim chunks
MAGIC = 12582912.0        # 1.5 * 2**23: fp32 round-to-nearest-even magic
INV_SQRT_HD = 1.0 / float(np.sqrt(HD))

_CACHED_NC = None


def build_program():
    nc = bacc.Bacc(
        "TRN2",
        target_bir_lowering=False,
        debug=False,
        enable_asserts=True,
        num_devices=NCORES,
    )
    rg = [list(range(NCORES))]

    # ---------------- I/O ----------------
    x_cols = nc.dram_tensor("x_cols", [R, OQ], F32, kind="ExternalInput").ap()
    wqkv_b = nc.dram_tensor("wqkv_b", [D, 3 * OQ], BF16, kind="ExternalInput").ap()
    wo_b = nc.dram_tensor("wo_b", [D, OQ], BF16, kind="ExternalInput").ap()
    wgu_b = nc.dram_tensor("wgu_b", [D, 2 * OM], BF16, kind="ExternalInput").ap()
    wd_b = nc.dram_tensor("wd_b", [MLP, OQ], BF16, kind="ExternalInput").ap()
    n1c_w = nc.dram_tensor("n1c_w", [1, OQ], F32, kind="ExternalInput").ap()
    n2c_w = nc.dram_tensor("n2c_w", [1, OQ], F32, kind="ExternalInput").ap()
    csc = nc.dram_tensor("csc", [1, 8], F32, kind="ExternalInput").ap()
    causal = nc.dram_tensor("causal", [P, P], F32, kind="ExternalInput").ap()
    out_d = nc.dram_tensor("out", [R, OQ], F32, kind="ExternalOutput").ap()

    with tile.TileContext(nc) as tc, \
         tc.tile_pool(name="persist", bufs=1) as pp, \
         tc.tile_pool(name="dram", bufs=1, space="DRAM") as dp:

        # ---------------- collective DRAM buffers (h = token half) ----------
        ssq1_in = dp.tile([R, 1], F32, tag="ssq1_in")
        ssq1_g = dp.tile([R, 1], F32, tag="ssq1_g", addr_space="Shared")
        am1_in = dp.tile([R, 1], F32, tag="am1_in")
        am1_g = dp.tile([R, 1], F32, tag="am1_g", addr_space="Shared")
        as1_d = dp.tile([R, 1], F32, tag="as1_d")
        ag1_in = [dp.tile([OQ, S], BF16, tag=f"ag1i{h}", name=f"ag1i{h}")
                  for h in range(2)]
        ag1_out = [dp.tile([D, S], BF16, tag=f"ag1o{h}", name=f"ag1o{h}",
                           addr_space="Shared") for h in range(2)]
        aso_in = [dp.tile([S, 1], F32, tag=f"asoi{h}", name=f"asoi{h}")
                  for h in range(2)]
        aso_g = [dp.tile([S, 1], F32, tag=f"asog{h}", name=f"asog{h}",
                         addr_space="Shared") for h in range(2)]
        ago_in = [dp.tile([OQ, S], BF16, tag=f"agoi{h}", name=f"agoi{h}")
                  for h in range(2)]
        ago_out = [dp.tile([D, S], BF16, tag=f"agoo{h}", name=f"agoo{h}",
                           addr_space="Shared") for h in range(2)]
        ssq2_in = [dp.tile([S, 1], F32, tag=f"sq2i{h}", name=f"sq2i{h}")
                   for h in range(2)]
        ssq2_g = [dp.tile([S, 1], F32, tag=f"sq2g{h}", name=f"sq2g{h}",
                          addr_space="Shared") for h in range(2)]
        am2_in = [dp.tile([S, 1], F32, tag=f"am2i{h}", name=f"am2i{h}")
                  for h in range(2)]
        am2_g = [dp.tile([S, 1], F32, tag=f"am2g{h}", name=f"am2g{h}",
                         addr_space="Shared") for h in range(2)]
        ag2_in = [dp.tile([OQ, S], BF16, tag=f"ag2i{h}", name=f"ag2i{h}")
                  for h in range(2)]
        ag2_out = [dp.tile([D, S], BF16, tag=f"ag2o{h}", name=f"ag2o{h}",
                           addr_space="Shared") for h in range(2)]
        asm_in = [dp.tile([4 * P, 1], F32, tag=f"asmi{g}", name=f"asmi{g}")
                  for g in range(4)]
        asm_g = [dp.tile([4 * P, 1], F32, tag=f"asmg{g}", name=f"asmg{g}",
                         addr_space="Shared") for g in range(4)]
        agm_in = [dp.tile([OM, S], BF16, tag=f"agmi{h}", name=f"agmi{h}")
                  for h in range(2)]
        agm_out = [dp.tile([MLP, S], BF16, tag=f"agmo{h}", name=f"agmo{h}",
                           addr_space="Shared") for h in range(2)]
        x1_d = dp.tile([R, OQ], F32, tag="x1_d")

        # ---------------- persistent small tiles ----------------
        mask = pp.tile([P, P], F32, tag="mask")
        nc.sync.dma_start(mask[:], causal)
        cscr = pp.tile([1, 8], F32, tag="cscr")
        nc.sync.dma_start(cscr[:], csc)
        cb = pp.tile([P, 8], F32, tag="cb")
        nc.gpsimd.partition_broadcast(cb[:], cscr[0:1, :])
        # cb cols: 0=c_qk 1=c_v 2=c_o 3=c_g 4=c_u 5=c_d

        n1c = pp.tile([P, OQ], F32, tag="n1c")
        n1r = pp.tile([1, OQ], F32, tag="n1r")
        nc.sync.dma_start(n1r[:], n1c_w)
        nc.gpsimd.partition_broadcast(n1c[:], n1r[0:1, :])
        n2c = pp.tile([P, OQ], F32, tag="n2c")
        n2r = pp.tile([1, OQ], F32, tag="n2r")
        nc.sync.dma_start(n2r[:], n2c_w)
        nc.gpsimd.partition_broadcast(n2c[:], n2r[0:1, :])

        sc = {}
        for nm in ["as1g", "cq", "scv", "f1", "asog", "qso", "sc_o", "f2",
                   "sc_g", "sc_u", "asmg", "qsm", "sc_d", "tmp1", "tmp2"]:
            sc[nm] = pp.tile([P, RT], F32, tag=nm, name=nm)

        def rn_scales(ssq_g_ap, am_g_ap, cols, f_out, as_out=None,
                      extra=()):
            """Load per-token AR'd ssq/amax, produce quant factor f =
            rinv*127/as and optional scale products into persist tiles."""
            rv = sc["tmp1"]
            nc.sync.dma_start(
                rv[:, cols], ssq_g_ap.rearrange("(t p) o -> p (t o)", p=P)
            )
            nc.vector.tensor_scalar(
                rv[:, cols], rv[:, cols], 1.0 / D, 1e-6,
                op0=ALU.mult, op1=ALU.add,
            )
            nc.scalar.activation(rv[:, cols], rv[:, cols], AF.Sqrt)
            nc.vector.reciprocal(rv[:, cols], rv[:, cols])   # rinv
            av = sc["tmp2"]
            nc.sync.dma_start(
                av[:, cols], am_g_ap.rearrange("(t p) o -> p (t o)", p=P)
            )
            nc.vector.tensor_tensor(
                av[:, cols], av[:, cols], rv[:, cols], op=ALU.mult
            )
            nc.vector.tensor_scalar(
                av[:, cols], av[:, cols], 1e-8, None, op0=ALU.add
            )  # a_scale
            if as_out is not None:
                nc.vector.tensor_copy(as_out[:, cols], av[:, cols])
            nc.vector.reciprocal(f_out[:, cols], av[:, cols])
            nc.vector.tensor_tensor(
                f_out[:, cols], f_out[:, cols], rv[:, cols], op=ALU.mult
            )
            nc.vector.tensor_scalar(
                f_out[:, cols], f_out[:, cols], 127.0, None, op0=ALU.mult
            )
            for dst, ci in extra:
                nc.vector.tensor_scalar(
                    dst[:, cols], av[:, cols], cb[:, ci:ci + 1], None,
                    op0=ALU.mult,
                )

        def col_quant_transpose(sp, src_ap, f_ap, nw_tile, tq_tag):
            """a_q = round(src * f * nw) bf16 [P, OQ]; return transposed
            [P, 2, P] tile (XBAR transpose on Sync)."""
            tmp = sp.tile([P, OQ], F32, tag=f"{tq_tag}_t", name=f"{tq_tag}_t")
            nc.vector.scalar_tensor_tensor(
                tmp[:], src_ap, f_ap, nw_tile[:], op0=ALU.mult, op1=ALU.mult
            )
            nc.vector.tensor_scalar(tmp[:], tmp[:], MAGIC, None, op0=ALU.add)
            a_q = sp.tile([P, OQ], BF16, tag=f"{tq_tag}_q", name=f"{tq_tag}_q")
            nc.scalar.activation(a_q[:], tmp[:], AF.Copy, bias=-MAGIC, scale=1.0)
            tq = sp.tile([P, 2, P], BF16, tag=f"{tq_tag}_x", name=f"{tq_tag}_x")
            nc.sync.dma_start(tq[:], a_q[:], transpose=True)
            return tq

        # =========================================================
        # pL1 spans phases 1-4: x column slices + x1 tiles
        # =========================================================
        with tc.tile_pool(name="pL1", bufs=1) as pL1:
            xc = [pL1.tile([P, OQ], F32, tag=f"xc{t}", name=f"xc{t}")
                  for t in range(RT)]
            x1 = [pL1.tile([P, OQ], F32, tag=f"x1_{t}", name=f"x1_{t}")
                  for t in range(RT)]

            # ---- Phase 1: local norm1 partials + AR + quant + AG halves ----
            with tc.tile_pool(name="p1s", bufs=2) as s1:
                for t in range(RT):
                    nc.scalar.dma_start(
                        xc[t][:], x_cols[t * P:(t + 1) * P, :]
                    )
                    sq = s1.tile([P, OQ], F32, tag="sq")
                    ssq_l = s1.tile([P, 1], F32, tag="ssq_l")
                    nc.scalar.activation(
                        sq[:], xc[t][:], AF.Square, accum_out=ssq_l[:]
                    )
                    hn = s1.tile([P, OQ], F32, tag="hn")
                    nc.vector.tensor_tensor(
                        hn[:], xc[t][:], n1c[:], op=ALU.mult
                    )
                    am_l = s1.tile([P, 1], F32, tag="am_l")
                    nc.vector.tensor_reduce(
                        am_l[:], hn[:], op=ALU.max, axis=AX.X,
                        apply_absolute_value=True,
                    )
                    nc.sync.dma_start(ssq1_in[t * P:(t + 1) * P, :], ssq_l[:])
                    nc.sync.dma_start(am1_in[t * P:(t + 1) * P, :], am_l[:])
            nc.gpsimd.collective_compute(
                "AllReduce", ALU.add, replica_groups=rg,
                ins=[ssq1_in.opt()], outs=[ssq1_g.opt()],
            )
            nc.gpsimd.collective_compute(
                "AllReduce", ALU.max, replica_groups=rg,
                ins=[am1_in.opt()], outs=[am1_g.opt()],
            )
            rn_scales(ssq1_g, am1_g, slice(0, RT), sc["f1"], sc["as1g"],
                      extra=((sc["cq"], 0), (sc["scv"], 1)))
            # as1 row layout for dkb (k-column scale in scores)
            nc.sync.dma_start(
                as1_d.rearrange("(t p) o -> p (t o)", p=P), sc["as1g"][:]
            )
            with tc.tile_pool(name="p1q", bufs=2) as s1q:
                for h in range(2):
                    for tl in range(ST):
                        t = h * ST + tl
                        tq = col_quant_transpose(
                            s1q, xc[t][:], sc["f1"][:, t:t + 1], n1c, "q1"
                        )
                        nc.scalar.dma_start(
                            ag1_in[h].rearrange("(kb p) r -> p kb r", p=P)
                            [:, :, tl * P:(tl + 1) * P],
                            tq[:],
                        )
                    nc.gpsimd.collective_compute(
                        "AllGather", ALU.bypass, replica_groups=rg,
                        ins=[ag1_in[h].opt()], outs=[ag1_out[h].opt()],
                    )

            # =====================================================
            # pL2 spans phases 2-4 (qkT/v through attention, wo to o-proj)
            # =====================================================
            with tc.tile_pool(name="pL2", bufs=1) as pL2:
                qkT = pL2.tile([P, 4, R], BF16, tag="qkT")
                v_sb = [pL2.tile([P, 2 * P], BF16, tag=f"v{t}", name=f"v{t}")
                        for t in range(RT)]
                wo_sb = [pL2.tile([P, OQ], BF16, tag=f"wo{k}", name=f"wo{k}")
                         for k in range(KT)]
                dkb = pL2.tile([P, R], F32, tag="dkb")
                dk_row = pL2.tile([1, R], F32, tag="dk_row")
                nc.sync.dma_start(dk_row[:], as1_d.rearrange("r o -> o r"))
                nc.gpsimd.partition_broadcast(dkb[:], dk_row[0:1, :])

                # ---- Phase 2: QKV (std layout + one transpose per tile) ----
                with tc.tile_pool(name="pL3", bufs=1) as pL3, \
                     tc.tile_pool(name="p2s", bufs=2) as s2, \
                     tc.tile_pool(name="ps2", bufs=2, space="PSUM") as ps2:
                    wqkv_sb = [pL3.tile([P, 3 * OQ], BF16, tag=f"wq{k}",
                                        name=f"wq{k}") for k in range(KT)]
                    for k in range(KT):
                        nc.scalar.dma_start(
                            wqkv_sb[k][:], wqkv_b[k * P:(k + 1) * P, :]
                        )
                    atile = [pL3.tile([P, S], BF16, tag=f"at{k}_{h}",
                                      name=f"at{k}_{h}")
                             for h in range(2) for k in range(KT)]

                    def at(k, h):
                        return atile[h * KT + k]

                    for h in range(2):
                        for k in range(KT):
                            nc.scalar.dma_start(
                                at(k, h)[:],
                                ag1_out[h][k * P:(k + 1) * P, :],
                            )
                        for tl in range(ST):
                            t = h * ST + tl
                            psq = ps2.tile([P, 3 * OQ], F32, tag="psq")
                            for kb in range(KT):
                                lhsT = at(kb, h)[:, tl * P:(tl + 1) * P]
                                nc.tensor.matmul(
                                    psq[:, 0:512], lhsT, wqkv_sb[kb][:, 0:512],
                                    start=(kb == 0), stop=(kb == KT - 1),
                                )
                                nc.tensor.matmul(
                                    psq[:, 512:768], lhsT,
                                    wqkv_sb[kb][:, 512:768],
                                    start=(kb == 0), stop=(kb == KT - 1),
                                )
                            qks = s2.tile([P, 512], BF16, tag="qks")
                            nc.vector.tensor_copy(qks[:], psq[:, 0:512])
                            nc.sync.dma_start(
                                qkT[:, :, t * P:(t + 1) * P], qks[:],
                                transpose=True,
                            )
                            nc.vector.tensor_scalar(
                                v_sb[t][:], psq[:, 512:768],
                                sc["scv"][:, t:t + 1], None, op0=ALU.mult,
                            )

                # o-proj weights prefetch (runs during attention)
                for k in range(KT):
                    nc.scalar.dma_start(
                        wo_sb[k][:], wo_b[k * P:(k + 1) * P, :]
                    )

                # ---- Phase 3: attention (per batch half) ----
                att_sb = [pL2.tile([P, 2 * P], F32, tag=f"att{t}",
                                   name=f"att{t}") for t in range(RT)]
                aso_sb = pL2.tile([P, RT], F32, tag="aso_sb")
                with tc.tile_pool(name="pAt", bufs=2) as sat, \
                     tc.tile_pool(name="ps3s", bufs=2, space="PSUM") as ps3s, \
                     tc.tile_pool(name="ps3a", bufs=4, space="PSUM") as ps3a, \
                     tc.tile_pool(name="pqa", bufs=2) as sqa:
                    for b in range(B):
                        for i in range(ST):
                            tg = b * ST + i
                            L = (i + 1) * P
                            for hl in range(2):
                                q_i, k_i = hl, 2 + hl
                                Spp = ps3s.tile([P, S], F32, tag="Spp")
                                lhsT = qkT[:, q_i, tg * P:(tg + 1) * P]
                                for jc in range((L + 511) // 512):
                                    n0 = jc * 512
                                    n1 = min(L, n0 + 512)
                                    nc.tensor.matmul(
                                        Spp[:, n0:n1], lhsT,
                                        qkT[:, k_i, b * S + n0:b * S + n1],
                                        start=True, stop=True,
                                    )
                                S1 = sat.tile([P, S], F32, tag="S1")
                                nc.vector.scalar_tensor_tensor(
                                    S1[:, 0:L], Spp[:, 0:L],
                                    sc["cq"][:, tg:tg + 1],
                                    dkb[:, b * S:b * S + L],
                                    op0=ALU.mult, op1=ALU.mult,
                                )
                                nc.vector.tensor_tensor(
                                    S1[:, i * P:L], S1[:, i * P:L], mask[:],
                                    op=ALU.add,
                                )
                                negmx = sat.tile([P, 1], F32, tag="negmx")
                                nc.vector.tensor_reduce(
                                    negmx[:], S1[:, 0:L], op=ALU.max,
                                    axis=AX.X, negate=True,
                                )
                                esum = sat.tile([P, 1], F32, tag="esum")
                                nc.scalar.activation(
                                    S1[:, 0:L], S1[:, 0:L], AF.Exp,
                                    bias=negmx[:, 0:1], scale=1.0,
                                    accum_out=esum[:],
                                )
                                erec = sat.tile([P, 1], F32, tag="erec")
                                nc.vector.reciprocal(erec[:], esum[:])
                                Pb = sat.tile([P, S], BF16, tag="Pb")
                                nc.vector.tensor_scalar(
                                    Pb[:, 0:L], S1[:, 0:L], erec[:, 0:1],
                                    None, op0=ALU.mult,
                                )
                                pts = sat.tile([P, ST, P], BF16, tag="pts")
                                nc.sync.dma_start(
                                    pts[:, 0:i + 1, :], Pb[:, 0:L],
                                    transpose=True,
                                )
                                att = ps3a.tile([P, P], F32, tag="att")
                                for j in range(i + 1):
                                    nc.tensor.matmul(
                                        att[:], pts[:, j, :],
                                        v_sb[b * ST + j]
                                        [:, hl * P:(hl + 1) * P],
                                        start=(j == 0), stop=(j == i),
                                    )
                                nc.vector.tensor_copy(
                                    att_sb[tg][:, hl * P:(hl + 1) * P],
                                    att[:],
                                )
                            nc.vector.tensor_reduce(
                                aso_sb[:, tg:tg + 1], att_sb[tg][:],
                                op=ALU.max, axis=AX.X,
                                apply_absolute_value=True,
                            )
                        # per-half attn-out scale + quant + AG
                        nc.sync.dma_start(
                            aso_in[b].rearrange("(t p) o -> p (t o)", p=P),
                            aso_sb[:, b * ST:(b + 1) * ST],
                        )
                        nc.gpsimd.collective_compute(
                            "AllReduce", ALU.max, replica_groups=rg,
                            ins=[aso_in[b].opt()], outs=[aso_g[b].opt()],
                        )
                        cols = slice(b * ST, (b + 1) * ST)
                        nc.sync.dma_start(
                            sc["asog"][:, cols],
                            aso_g[b].rearrange("(t p) o -> p (t o)", p=P),
                        )
                        nc.vector.tensor_scalar(
                            sc["asog"][:, cols], sc["asog"][:, cols], 1e-8,
                            None, op0=ALU.add,
                        )
                        nc.vector.reciprocal(
                            sc["qso"][:, cols], sc["asog"][:, cols]
                        )
                        nc.vector.tensor_scalar(
                            sc["qso"][:, cols], sc["qso"][:, cols], 127.0,
                            None, op0=ALU.mult,
                        )
                        nc.vector.tensor_scalar(
                            sc["sc_o"][:, cols], sc["asog"][:, cols],
                            cb[:, 2:3], None, op0=ALU.mult,
                        )
                        for tl in range(ST):
                            t = b * ST + tl
                            tmp = sqa.tile([P, 2 * P], F32, tag="qat")
                            nc.vector.tensor_scalar(
                                tmp[:], att_sb[t][:], sc["qso"][:, t:t + 1],
                                MAGIC, op0=ALU.mult, op1=ALU.add,
                            )
                            a_qo = sqa.tile([P, 2 * P], BF16, tag="a_qo")
                            nc.scalar.activation(
                                a_qo[:], tmp[:], AF.Copy, bias=-MAGIC,
                                scale=1.0,
                            )
                            tqa = sqa.tile([P, 2, P], BF16, tag="tqa")
                            nc.sync.dma_start(
                                tqa[:], a_qo[:], transpose=True
                            )
                            nc.scalar.dma_start(
                                ago_in[b].rearrange("(kb p) r -> p kb r", p=P)
                                [:, :, tl * P:(tl + 1) * P],
                                tqa[:],
                            )
                        nc.gpsimd.collective_compute(
                            "AllGather", ALU.bypass, replica_groups=rg,
                            ins=[ago_in[b].opt()], outs=[ago_out[b].opt()],
                        )

                # ---- Phase 4: o-proj + residual + norm2 + AG2 (halves) ----
                with tc.tile_pool(name="pO", bufs=1) as pO, \
                     tc.tile_pool(name="psO", bufs=4, space="PSUM") as psO, \
                     tc.tile_pool(name="p4s", bufs=2) as s4, \
                     tc.tile_pool(name="pq2", bufs=2) as sq2:
                    oT = [pO.tile([P, S], BF16, tag=f"oT{k}_{h}",
                                  name=f"oT{k}_{h}")
                          for h in range(2) for k in range(KT)]

                    def ot_(k, h):
                        return oT[h * KT + k]

                    for h in range(2):
                        for k in range(KT):
                            nc.scalar.dma_start(
                                ot_(k, h)[:],
                                ago_out[h][k * P:(k + 1) * P, :],
                            )
                        for tl in range(ST):
                            t = h * ST + tl
                            pso = psO.tile([P, OQ], F32, tag="pso")
                            for kb in range(KT):
                                nc.tensor.matmul(
                                    pso[:],
                                    ot_(kb, h)[:, tl * P:(tl + 1) * P],
                                    wo_sb[kb][:],
                                    start=(kb == 0), stop=(kb == KT - 1),
                                )
                            nc.vector.scalar_tensor_tensor(
                                x1[t][:], pso[:], sc["sc_o"][:, t:t + 1],
                                xc[t][:], op0=ALU.mult, op1=ALU.add,
                            )
                            nc.sync.dma_start(
                                x1_d[t * P:(t + 1) * P, :], x1[t][:]
                            )
                            sq = s4.tile([P, OQ], F32, tag="sq4")
                            ssq_l = s4.tile([P, 1], F32, tag="ssq_l4")
                            nc.scalar.activation(
                                sq[:], x1[t][:], AF.Square,
                                accum_out=ssq_l[:],
                            )
                            hn = s4.tile([P, OQ], F32, tag="hn4")
                            nc.vector.tensor_tensor(
                                hn[:], x1[t][:], n2c[:], op=ALU.mult
                            )
                            am_l = s4.tile([P, 1], F32, tag="am_l4")
                            nc.vector.tensor_reduce(
                                am_l[:], hn[:], op=ALU.max, axis=AX.X,
                                apply_absolute_value=True,
                            )
                            nc.sync.dma_start(
                                ssq2_in[h][tl * P:(tl + 1) * P, :], ssq_l[:]
                            )
                            nc.sync.dma_start(
                                am2_in[h][tl * P:(tl + 1) * P, :], am_l[:]
                            )
                        nc.gpsimd.collective_compute(
                            "AllReduce", ALU.add, replica_groups=rg,
                            ins=[ssq2_in[h].opt()], outs=[ssq2_g[h].opt()],
                        )
                        nc.gpsimd.collective_compute(
                            "AllReduce", ALU.max, replica_groups=rg,
                            ins=[am2_in[h].opt()], outs=[am2_g[h].opt()],
                        )
                        cols = slice(h * ST, (h + 1) * ST)
                        rn_scales(ssq2_g[h], am2_g[h], cols, sc["f2"],
                                  extra=((sc["sc_g"], 3), (sc["sc_u"], 4)))
                        for tl in range(ST):
                            t = h * ST + tl
                            tq = col_quant_transpose(
                                sq2, x1[t][:], sc["f2"][:, t:t + 1], n2c,
                                "q2",
                            )
                            nc.scalar.dma_start(
                                ag2_in[h].rearrange("(kb p) r -> p kb r", p=P)
                                [:, :, tl * P:(tl + 1) * P],
                                tq[:],
                            )
                        nc.gpsimd.collective_compute(
                            "AllGather", ALU.bypass, replica_groups=rg,
                            ins=[ag2_in[h].opt()], outs=[ag2_out[h].opt()],
                        )

        # ---- Phase 5: MLP gate/up + m quant + AG(mT) halves ----
        with tc.tile_pool(name="pM", bufs=1) as pM:
            wgu_sb = [pM.tile([P, 2 * OM], BF16, tag=f"wgu{k}", name=f"wgu{k}")
                      for k in range(KT)]
            for k in range(KT):
                nc.scalar.dma_start(wgu_sb[k][:], wgu_b[k * P:(k + 1) * P, :])
            a2t = [pM.tile([P, S], BF16, tag=f"a2t{k}_{h}", name=f"a2t{k}_{h}")
                   for h in range(2) for k in range(KT)]

            def a2(k, h):
                return a2t[h * KT + k]

            asm_sb = pM.tile([P, RT], F32, tag="asm_sb")
            with tc.tile_pool(name="p5s", bufs=2) as s5, \
                 tc.tile_pool(name="ps5", bufs=2, space="PSUM") as ps5:
                for h in range(2):
                    for k in range(KT):
                        nc.scalar.dma_start(
                            a2(k, h)[:], ag2_out[h][k * P:(k + 1) * P, :]
                        )
                    for gl in range(2):
                        g = h * 2 + gl
                        m_tiles = {}
                        for tl4 in range(4):
                            tl = gl * 4 + tl4
                            t = h * ST + tl
                            psg = ps5.tile([P, OM], F32, tag="psg")
                            psu = ps5.tile([P, OM], F32, tag="psu")
                            for kb in range(KT):
                                lhsT = a2(kb, h)[:, tl * P:(tl + 1) * P]
                                for n in range(2):
                                    nc.tensor.matmul(
                                        psg[:, n * 512:(n + 1) * 512], lhsT,
                                        wgu_sb[kb][:, n * 512:(n + 1) * 512],
                                        start=(kb == 0), stop=(kb == KT - 1),
                                    )
                            for kb in range(KT):
                                lhsT = a2(kb, h)[:, tl * P:(tl + 1) * P]
                                for n in range(2):
                                    nc.tensor.matmul(
                                        psu[:, n * 512:(n + 1) * 512], lhsT,
                                        wgu_sb[kb]
                                        [:, OM + n * 512:OM + (n + 1) * 512],
                                        start=(kb == 0), stop=(kb == KT - 1),
                                    )
                            sig = s5.tile([P, OM], F32, tag="sig")
                            nc.scalar.activation(
                                sig[:], psg[:], AF.Sigmoid,
                                scale=sc["sc_g"][:, t:t + 1],
                            )
                            sgl = s5.tile([P, OM], F32, tag="sgl")
                            nc.vector.scalar_tensor_tensor(
                                sgl[:], psg[:], sc["sc_g"][:, t:t + 1],
                                sig[:], op0=ALU.mult, op1=ALU.mult,
                            )
                            mt = s5.tile([P, OM], F32, tag=f"m{tl4}",
                                         name=f"m{tl4}", bufs=2)
                            m_tiles[tl4] = mt
                            nc.vector.scalar_tensor_tensor(
                                mt[:], psu[:], sc["sc_u"][:, t:t + 1],
                                sgl[:], op0=ALU.mult, op1=ALU.mult,
                            )
                            nc.vector.tensor_reduce(
                                asm_sb[:, t:t + 1], mt[:], op=ALU.max,
                                axis=AX.X, apply_absolute_value=True,
                            )
                        nc.sync.dma_start(
                            asm_in[g].rearrange("(t p) o -> p (t o)", p=P),
                            asm_sb[:, g * 4:(g + 1) * 4],
                        )
                        nc.gpsimd.collective_compute(
                            "AllReduce", ALU.max, replica_groups=rg,
                            ins=[asm_in[g].opt()], outs=[asm_g[g].opt()],
                        )
                        gs = slice(g * 4, (g + 1) * 4)
                        nc.sync.dma_start(
                            sc["asmg"][:, gs],
                            asm_g[g].rearrange("(t p) o -> p (t o)", p=P),
                        )
                        nc.vector.tensor_scalar(
                            sc["asmg"][:, gs], sc["asmg"][:, gs], 1e-8,
                            None, op0=ALU.add,
                        )
                        nc.vector.reciprocal(
                            sc["qsm"][:, gs], sc["asmg"][:, gs]
                        )
                        nc.vector.tensor_scalar(
                            sc["qsm"][:, gs], sc["qsm"][:, gs], 127.0, None,
                            op0=ALU.mult,
                        )
                        nc.vector.tensor_scalar(
                            sc["sc_d"][:, gs], sc["asmg"][:, gs],
                            cb[:, 5:6], None, op0=ALU.mult,
                        )
                        for tl4 in range(4):
                            tl = gl * 4 + tl4
                            t = h * ST + tl
                            tmp = s5.tile([P, OM], F32, tag="qm")
                            nc.vector.tensor_scalar(
                                tmp[:], m_tiles[tl4][:],
                                sc["qsm"][:, t:t + 1], MAGIC,
                                op0=ALU.mult, op1=ALU.add,
                            )
                            m_q = s5.tile([P, OM], BF16, tag="m_q")
                            nc.scalar.activation(
                                m_q[:], tmp[:], AF.Copy, bias=-MAGIC,
                                scale=1.0,
                            )
                            tqm = s5.tile([P, KM, P], BF16, tag="tqm")
                            nc.sync.dma_start(tqm[:], m_q[:], transpose=True)
                            nc.scalar.dma_start(
                                agm_in[h].rearrange("(kb p) r -> p kb r", p=P)
                                [:, :, tl * P:(tl + 1) * P],
                                tqm[:],
                            )
                    nc.gpsimd.collective_compute(
                        "AllGather", ALU.bypass, replica_groups=rg,
                        ins=[agm_in[h].opt()], outs=[agm_out[h].opt()],
                    )

        # ---- Phase 6: down (column-parallel, full-MLP contraction) ----
        with tc.tile_pool(name="pD", bufs=1) as pD, \
             tc.tile_pool(name="p6s", bufs=3) as s6, \
             tc.tile_pool(name="ps6", bufs=1, space="PSUM") as ps6:
            wd_sb = [pD.tile([P, OQ], BF16, tag=f"wd{k}", name=f"wd{k}")
                     for k in range(MLP // P)]
            for k in range(MLP // P):
                nc.scalar.dma_start(wd_sb[k][:], wd_b[k * P:(k + 1) * P, :])
            for hh in range(2):
                psd = [ps6.tile([P, OQ], F32, tag=f"psd{tl}", name=f"psd{tl}")
                       for tl in range(ST)]
                for kb in range(MLP // P):
                    mt_l = s6.tile([P, S], BF16, tag="mt_l")
                    nc.scalar.dma_start(
                        mt_l[:], agm_out[hh][kb * P:(kb + 1) * P, :]
                    )
                    for tl in range(ST):
                        nc.tensor.matmul(
                            psd[tl][:], mt_l[:, tl * P:(tl + 1) * P],
                            wd_sb[kb][:],
                            start=(kb == 0), stop=(kb == MLP // P - 1),
                        )
                for tl in range(ST):
                    t = hh * ST + tl
                    x1r = s6.tile([P, OQ], F32, tag="x1r")
                    nc.scalar.dma_start(
                        x1r[:], x1_d[t * P:(t + 1) * P, :]
                    )
                    ot2 = s6.tile([P, OQ], F32, tag="ot2")
                    nc.vector.scalar_tensor_tensor(
                        ot2[:], psd[tl][:], sc["sc_d"][:, t:t + 1], x1r[:],
                        op0=ALU.mult, op1=ALU.add,
                    )
                    nc.sync.dma_start(out_d[t * P:(t + 1) * P, :], ot2[:])

    nc.compile()
    return nc


def _prep_in_maps(inputs):
    x = np.asarray(inputs["x"], np.float32).reshape(R, D)
    wq = np.asarray(inputs["wq"], np.float32)
    wk = np.asarray(inputs["wk"], np.float32)
    wv = np.asarray(inputs["wv"], np.float32)
    wo = np.asarray(inputs["wo"], np.float32)
    wg = np.asarray(inputs["wg"], np.float32)
    wu = np.asarray(inputs["wu"], np.float32)
    wd = np.asarray(inputs["wd"], np.float32)
    n1 = np.asarray(inputs["norm1_w"], np.float32).reshape(1, D)
    n2 = np.asarray(inputs["norm2_w"], np.float32).reshape(1, D)

    def wscale(w):
        return float(np.abs(w.astype(np.float64)).mean()) + 1e-8

    def tern(w, ws):
        return np.clip(np.round(w / np.float32(ws)), -1.0, 1.0) \
            .astype(ml_dtypes.bfloat16)

    ws_q, ws_k, ws_v = wscale(wq), wscale(wk), wscale(wv)
    ws_o, ws_g, ws_u, ws_d = wscale(wo), wscale(wg), wscale(wu), wscale(wd)
    wq_t = tern(wq, ws_q)
    wk_t = tern(wk, ws_k)
    wv_t = tern(wv, ws_v)
    wo_t = tern(wo, ws_o)
    wg_t = tern(wg, ws_g)
    wu_t = tern(wu, ws_u)
    wd_t = tern(wd, ws_d)

    csc = np.array([[
        ws_q * ws_k * INV_SQRT_HD / (127.0 * 127.0),
        ws_v / 127.0, ws_o / 127.0, ws_g / 127.0, ws_u / 127.0,
        ws_d / 127.0, 0.0, 0.0,
    ]], np.float32)
    iv, jv = np.mgrid[0:P, 0:P]
    causal = np.where(jv <= iv, 0.0, -1e30).astype(np.float32)

    in_maps = []
    for c in range(NCORES):
        qs = slice(c * OQ, (c + 1) * OQ)
        ms = slice(c * OM, (c + 1) * OM)
        in_maps.append({
            "x_cols": np.ascontiguousarray(x[:, qs]),
            "wqkv_b": np.ascontiguousarray(
                np.concatenate([wq_t[qs], wk_t[qs], wv_t[qs]], 0).T
            ),
            "wo_b": np.ascontiguousarray(wo_t[qs].T),
            "wgu_b": np.ascontiguousarray(
                np.concatenate([wg_t[ms], wu_t[ms]], 0).T
            ),
            "wd_b": np.ascontiguousarray(wd_t[qs].T),
            "n1c_w": np.ascontiguousarray(n1[:, qs]),
            "n2c_w": np.ascontiguousarray(n2[:, qs]),
            "csc": csc,
            "causal": causal,
        })
    return in_maps


def _assemble(results) -> np.ndarray:
    out = np.empty((R, D), np.float32)
    for c in range(NCORES):
        out[:, c * OQ:(c + 1) * OQ] = results[c]["out"]
    return out.reshape(B, S, D)


def kernel(**inputs) -> np.ndarray:
    global _CACHED_NC
    if _CACHED_NC is None:
        _CACHED_NC = build_program()
    nc = _CACHED_NC
    in_maps = _prep_in_maps(inputs)
    res = run_bass_kernel_spmd(nc, in_maps, core_ids=list(range(NCORES)))
    return _assemble(res.results).astype(np.float32)


# revision 10
# speedup vs baseline: 1.1899x; 1.1899x over previous
"""BitNet transformer block on 8 Trainium2 NeuronCores — v4.

Host-quantized ternary weights (bf16, exact); fully column-parallel matmuls;
all cross-core activations travel TRANSPOSED [feature, token] through small
bf16 AllGathers; rmsnorm stats / quant scales are [tokens,1] AllReduces; no
ReduceScatter.  The final output is column-sharded [R, 256] per core and the
host concatenates.

v4 scheduling notes (engine queues are FIFO in emission order):
  - weight loads are emitted before dependent compute on the Scalar queue;
  - quantization casts run on Vector so the Scalar queue (exp/sigmoid) is
    never blocked behind an AllReduce wait;
  - batch-0's attention-output quant + AllGather is emitted in the middle of
    batch-1's attention so the collective overlaps compute;
  - all XBAR transposes on Sync only (concurrent transposes corrupt data);
  - wgu/wd weight pools are reserved at top level so their loads are not
    space-gated on earlier phases.
"""

import numpy as np
import ml_dtypes

import concourse.bacc as bacc
import concourse.mybir as mybir
import concourse.tile as tile
from concourse.bass_utils import run_bass_kernel_spmd

F32 = mybir.dt.float32
BF16 = mybir.dt.bfloat16
AF = mybir.ActivationFunctionType
ALU = mybir.AluOpType
AX = mybir.AxisListType

NCORES = 8
B, S, D, H, MLP = 2, 1024, 2048, 16, 8192
HD = 128
R = B * S                 # 2048 token rows total
RL = R // NCORES          # 256 rows per core (phase-1 row shard)
OQ = D // NCORES          # 256 d-cols per core (2 heads)
OM = MLP // NCORES        # 1024 mlp cols per core
P = 128
KT = D // P               # 16 feature chunks
RT = R // P               # 16 token tiles
LT = RL // P              # 2 local row tiles
ST = S // P               # 8 token tiles per half (= batch)
KM = OM // P              # 8 local mlp-dim chunks
MAGIC = 12582912.0        # 1.5 * 2**23: fp32 round-to-nearest-even magic
INV_SQRT_HD = 1.0 / float(np.sqrt(HD))

_CACHED_NC = None


def build_program():
    nc = bacc.Bacc(
        "TRN2",
        target_bir_lowering=False,
        debug=False,
        enable_asserts=True,
        num_devices=NCORES,
    )
    rg = [list(range(NCORES))]

    # ---------------- I/O ----------------
    x_rows = nc.dram_tensor("x_rows", [RL, D], F32, kind="ExternalInput").ap()
    x_cols = nc.dram_tensor("x_cols", [R, OQ], F32, kind="ExternalInput").ap()
    wqkv_b = nc.dram_tensor("wqkv_b", [D, 3 * OQ], BF16, kind="ExternalInput").ap()
    wo_b = nc.dram_tensor("wo_b", [D, OQ], BF16, kind="ExternalInput").ap()
    wgu_b = nc.dram_tensor("wgu_b", [D, 2 * OM], BF16, kind="ExternalInput").ap()
    wd_b = nc.dram_tensor("wd_b", [MLP, OQ], BF16, kind="ExternalInput").ap()
    norm1_w = nc.dram_tensor("norm1_w", [1, D], F32, kind="ExternalInput").ap()
    n2c_w = nc.dram_tensor("n2c_w", [1, OQ], F32, kind="ExternalInput").ap()
    csc = nc.dram_tensor("csc", [1, 8], F32, kind="ExternalInput").ap()
    causal = nc.dram_tensor("causal", [P, P], F32, kind="ExternalInput").ap()
    out_d = nc.dram_tensor("out", [R, OQ], F32, kind="ExternalOutput").ap()

    with tile.TileContext(nc) as tc, \
         tc.tile_pool(name="persist", bufs=1) as pp, \
         tc.tile_pool(name="dram", bufs=1, space="DRAM") as dp:

        # ---------------- collective DRAM buffers (h = token half) ----------
        ag1_in = dp.tile([RL, D], BF16, tag="ag1_in")
        ag1_out = dp.tile([R, D], BF16, tag="ag1_out", addr_space="Shared")
        ag1s_in = dp.tile([RL, 1], F32, tag="ag1s_in")
        ag1s_out = dp.tile([R, 1], F32, tag="ag1s_out", addr_space="Shared")
        aso_in = [dp.tile([S, 1], F32, tag=f"asoi{h}", name=f"asoi{h}")
                  for h in range(2)]
        aso_g = [dp.tile([S, 1], F32, tag=f"asog{h}", name=f"asog{h}",
                         addr_space="Shared") for h in range(2)]
        ago_in = [dp.tile([OQ, S], BF16, tag=f"agoi{h}", name=f"agoi{h}")
                  for h in range(2)]
        ago_out = [dp.tile([D, S], BF16, tag=f"agoo{h}", name=f"agoo{h}",
                           addr_space="Shared") for h in range(2)]
        ssq2_in = [dp.tile([S, 1], F32, tag=f"sq2i{h}", name=f"sq2i{h}")
                   for h in range(2)]
        ssq2_g = [dp.tile([S, 1], F32, tag=f"sq2g{h}", name=f"sq2g{h}",
                          addr_space="Shared") for h in range(2)]
        am2_in = [dp.tile([S, 1], F32, tag=f"am2i{h}", name=f"am2i{h}")
                  for h in range(2)]
        am2_g = [dp.tile([S, 1], F32, tag=f"am2g{h}", name=f"am2g{h}",
                         addr_space="Shared") for h in range(2)]
        ag2_in = [dp.tile([OQ, S], BF16, tag=f"ag2i{h}", name=f"ag2i{h}")
                  for h in range(2)]
        ag2_out = [dp.tile([D, S], BF16, tag=f"ag2o{h}", name=f"ag2o{h}",
                           addr_space="Shared") for h in range(2)]
        asm_in = [dp.tile([4 * P, 1], F32, tag=f"asmi{g}", name=f"asmi{g}")
                  for g in range(4)]
        asm_g = [dp.tile([4 * P, 1], F32, tag=f"asmg{g}", name=f"asmg{g}",
                         addr_space="Shared") for g in range(4)]
        agm_in = [dp.tile([OM, S], BF16, tag=f"agmi{h}", name=f"agmi{h}")
                  for h in range(2)]
        agm_out = [dp.tile([MLP, S], BF16, tag=f"agmo{h}", name=f"agmo{h}",
                           addr_space="Shared") for h in range(2)]
        x1_d = dp.tile([R, OQ], F32, tag="x1_d")

        # ---------------- persistent small tiles ----------------
        mask = pp.tile([P, P], F32, tag="mask")
        nc.sync.dma_start(mask[:], causal)
        cscr = pp.tile([1, 8], F32, tag="cscr")
        nc.sync.dma_start(cscr[:], csc)
        cb = pp.tile([P, 8], F32, tag="cb")
        nc.gpsimd.partition_broadcast(cb[:], cscr[0:1, :])
        # cb cols: 0=c_qk 1=c_v 2=c_o 3=c_g 4=c_u 5=c_d

        n2c = pp.tile([P, OQ], F32, tag="n2c")
        n2r = pp.tile([1, OQ], F32, tag="n2r")
        nc.sync.dma_start(n2r[:], n2c_w)
        nc.gpsimd.partition_broadcast(n2c[:], n2r[0:1, :])

        sc = {}
        for nm in ["as1g", "cq", "scv", "asog", "qso", "sc_o", "f2",
                   "sc_g", "sc_u", "asmg", "qsm", "sc_d", "tmp1", "tmp2"]:
            sc[nm] = pp.tile([P, RT], F32, tag=nm, name=nm)

        # =====================================================
        # Top-level weight pool: reserved space so big weight loads are
        # never space-gated behind earlier phases.
        # =====================================================
        with tc.tile_pool(name="pWG", bufs=1) as pWG:
            wgu_sb = [pWG.tile([P, 2 * OM], BF16, tag=f"wgu{k}",
                               name=f"wgu{k}") for k in range(KT)]

            with tc.tile_pool(name="pL1", bufs=1) as pL1:
                xc = [pL1.tile([P, OQ], F32, tag=f"xc{t}", name=f"xc{t}")
                      for t in range(RT)]
                x1 = [pL1.tile([P, OQ], F32, tag=f"x1_{t}", name=f"x1_{t}")
                      for t in range(RT)]

                # ---- Phase 1: local rmsnorm1 + quant + transpose + AG ----
                with tc.tile_pool(name="p1s", bufs=2) as s1, \
                     tc.tile_pool(name="ps0", bufs=1, space="PSUM") as ps0:
                    nw1 = s1.tile([P, D], F32, tag="nw1", bufs=1)
                    nw1r = s1.tile([1, D], F32, tag="nw1r", bufs=1)
                    nc.sync.dma_start(nw1r[:], norm1_w)
                    nc.gpsimd.partition_broadcast(nw1[:], nw1r[0:1, :])
                    for lt in range(LT):
                        xt = s1.tile([P, D], F32, tag="xt")
                        nc.scalar.dma_start(
                            xt[:], x_rows[lt * P:(lt + 1) * P, :]
                        )
                        # rmsnorm + absmax + quant (all local)
                        sqd = ps0.tile([P, D], F32, tag="sqd")
                        ssq = s1.tile([P, 1], F32, tag="ssq")
                        nc.scalar.activation(
                            sqd[:], xt[:], AF.Square, accum_out=ssq[:]
                        )
                        rms = s1.tile([P, 1], F32, tag="rms")
                        nc.vector.tensor_scalar(
                            rms[:], ssq[:], 1.0 / D, 1e-6,
                            op0=ALU.mult, op1=ALU.add,
                        )
                        nc.scalar.activation(rms[:], rms[:], AF.Sqrt)
                        rinv = s1.tile([P, 1], F32, tag="rinv")
                        nc.vector.reciprocal(rinv[:], rms[:])
                        nc.vector.tensor_tensor(
                            xt[:], xt[:], nw1[:], op=ALU.mult
                        )
                        amax = s1.tile([P, 1], F32, tag="amax")
                        nc.vector.tensor_reduce(
                            amax[:], xt[:], op=ALU.max, axis=AX.X,
                            apply_absolute_value=True,
                        )
                        as_l = s1.tile([P, 1], F32, tag="as_l")
                        nc.vector.tensor_scalar(
                            as_l[:], amax[:], rinv[:, 0:1], 1e-8,
                            op0=ALU.mult, op1=ALU.add,
                        )
                        nc.sync.dma_start(
                            ag1s_in[lt * P:(lt + 1) * P, :], as_l[:]
                        )
                        qs = s1.tile([P, 1], F32, tag="qs")
                        nc.vector.reciprocal(qs[:], as_l[:])
                        nc.vector.tensor_scalar(
                            qs[:], qs[:], rinv[:, 0:1], 127.0,
                            op0=ALU.mult, op1=ALU.mult,
                        )
                        aq = s1.tile([P, D], BF16, tag="aq")
                        for c0 in (0, 1024):
                            tmp = s1.tile([P, 1024], F32, tag="qtmp")
                            nc.vector.tensor_scalar(
                                tmp[:], xt[:, c0:c0 + 1024], qs[:, 0:1],
                                MAGIC, op0=ALU.mult, op1=ALU.add,
                            )
                            nc.vector.tensor_scalar(
                                aq[:, c0:c0 + 1024], tmp[:], -MAGIC, None,
                                op0=ALU.add,
                            )
                        tq1 = s1.tile([P, KT, P], BF16, tag="tq1")
                        nc.sync.dma_start(tq1[:], aq[:], transpose=True)
                        nc.sync.dma_start(
                            ag1_in[lt * P:(lt + 1) * P, :]
                            .rearrange("p (kb q) -> p kb q", q=P),
                            tq1[:],
                        )
                nc.gpsimd.collective_compute(
                    "AllGather", ALU.bypass, replica_groups=rg,
                    ins=[ag1_in.opt()], outs=[ag1_out.opt()],
                )
                nc.gpsimd.collective_compute(
                    "AllGather", ALU.bypass, replica_groups=rg,
                    ins=[ag1s_in.opt()], outs=[ag1s_out.opt()],
                )

                # =====================================================
                # pL2 spans phases 2-4
                # =====================================================
                with tc.tile_pool(name="pL2", bufs=1) as pL2:
                    qkT = pL2.tile([P, 4, R], BF16, tag="qkT")
                    v_sb = [pL2.tile([P, 2 * P], BF16, tag=f"v{t}",
                                     name=f"v{t}") for t in range(RT)]
                    wo_sb = [pL2.tile([P, OQ], BF16, tag=f"wo{k}",
                                      name=f"wo{k}") for k in range(KT)]
                    dkb = pL2.tile([P, R], F32, tag="dkb")

                    # scale prep (after AG1s)
                    nc.sync.dma_start(
                        sc["as1g"][:],
                        ag1s_out.rearrange("(t p) o -> p (t o)", p=P),
                    )
                    nc.vector.tensor_scalar(
                        sc["cq"][:], sc["as1g"][:], cb[:, 0:1], None,
                        op0=ALU.mult,
                    )
                    nc.vector.tensor_scalar(
                        sc["scv"][:], sc["as1g"][:], cb[:, 1:2], None,
                        op0=ALU.mult,
                    )
                    dk_row = pL2.tile([1, R], F32, tag="dk_row")
                    nc.sync.dma_start(
                        dk_row[:], ag1s_out.rearrange("r o -> o r")
                    )
                    nc.gpsimd.partition_broadcast(dkb[:], dk_row[0:1, :])

                    # ---- Phase 2: QKV ----
                    with tc.tile_pool(name="pL3", bufs=1) as pL3, \
                         tc.tile_pool(name="p2s", bufs=2) as s2, \
                         tc.tile_pool(name="ps2", bufs=2, space="PSUM") as ps2:
                        wqkv_sb = [pL3.tile([P, 3 * OQ], BF16, tag=f"wq{k}",
                                            name=f"wq{k}") for k in range(KT)]
                        for k in range(KT):
                            nc.scalar.dma_start(
                                wqkv_sb[k][:], wqkv_b[k * P:(k + 1) * P, :]
                            )
                        for t in range(RT):
                            atile = pL3.tile([P, D], BF16, tag="atile",
                                             bufs=4)
                            nc.scalar.dma_start(
                                atile[:], ag1_out[t * P:(t + 1) * P, :]
                            )
                            psq = ps2.tile([P, 3 * OQ], F32, tag="psq")
                            for kb in range(KT):
                                lhsT = atile[:, kb * P:(kb + 1) * P]
                                nc.tensor.matmul(
                                    psq[:, 0:512], lhsT,
                                    wqkv_sb[kb][:, 0:512],
                                    start=(kb == 0), stop=(kb == KT - 1),
                                )
                                nc.tensor.matmul(
                                    psq[:, 512:768], lhsT,
                                    wqkv_sb[kb][:, 512:768],
                                    start=(kb == 0), stop=(kb == KT - 1),
                                )
                            qks = s2.tile([P, 512], BF16, tag="qks")
                            nc.vector.tensor_copy(qks[:], psq[:, 0:512])
                            nc.sync.dma_start(
                                qkT[:, :, t * P:(t + 1) * P], qks[:],
                                transpose=True,
                            )
                            nc.vector.tensor_scalar(
                                v_sb[t][:], psq[:, 512:768],
                                sc["scv"][:, t:t + 1], None, op0=ALU.mult,
                            )

                    # o-proj weights + x column slices prefetch
                    for k in range(KT):
                        nc.scalar.dma_start(
                            wo_sb[k][:], wo_b[k * P:(k + 1) * P, :]
                        )
                    for t in range(RT):
                        nc.scalar.dma_start(
                            xc[t][:], x_cols[t * P:(t + 1) * P, :]
                        )
                    # wgu prefetch (reserved space; loads run during attn)
                    for k in range(KT):
                        nc.scalar.dma_start(
                            wgu_sb[k][:], wgu_b[k * P:(k + 1) * P, :]
                        )

                    # ---- Phase 3: attention ----
                    with tc.tile_pool(name="pAs", bufs=1) as pAs, \
                         tc.tile_pool(name="pAt", bufs=2) as sat, \
                         tc.tile_pool(name="ps3s", bufs=2, space="PSUM") as ps3s, \
                         tc.tile_pool(name="ps3a", bufs=4, space="PSUM") as ps3a, \
                         tc.tile_pool(name="pqa", bufs=2) as sqa:
                        att_sb = [pAs.tile([P, 2 * P], F32, tag=f"att{t}",
                                           name=f"att{t}") for t in range(RT)]
                        aso_sb = pAs.tile([P, RT], F32, tag="aso_sb")

                        def attn_tail(b):
                            """quant + transpose + AG of batch-b attn out."""
                            cols = slice(b * ST, (b + 1) * ST)
                            nc.sync.dma_start(
                                sc["asog"][:, cols],
                                aso_g[b].rearrange("(t p) o -> p (t o)", p=P),
                            )
                            nc.vector.tensor_scalar(
                                sc["asog"][:, cols], sc["asog"][:, cols],
                                1e-8, None, op0=ALU.add,
                            )
                            nc.vector.reciprocal(
                                sc["qso"][:, cols], sc["asog"][:, cols]
                            )
                            nc.vector.tensor_scalar(
                                sc["qso"][:, cols], sc["qso"][:, cols],
                                127.0, None, op0=ALU.mult,
                            )
                            nc.vector.tensor_scalar(
                                sc["sc_o"][:, cols], sc["asog"][:, cols],
                                cb[:, 2:3], None, op0=ALU.mult,
                            )
                            for tl in range(ST):
                                t = b * ST + tl
                                tmp = sqa.tile([P, 2 * P], F32, tag="qat")
                                nc.vector.tensor_scalar(
                                    tmp[:], att_sb[t][:],
                                    sc["qso"][:, t:t + 1], MAGIC,
                                    op0=ALU.mult, op1=ALU.add,
                                )
                                a_qo = sqa.tile([P, 2 * P], BF16, tag="a_qo")
                                nc.vector.tensor_scalar(
                                    a_qo[:], tmp[:], -MAGIC, None,
                                    op0=ALU.add,
                                )
                                tqa = sqa.tile([P, 2, P], BF16, tag="tqa")
                                nc.sync.dma_start(
                                    tqa[:], a_qo[:], transpose=True
                                )
                                nc.sync.dma_start(
                                    ago_in[b]
                                    .rearrange("(kb p) r -> p kb r", p=P)
                                    [:, :, tl * P:(tl + 1) * P],
                                    tqa[:],
                                )
                            nc.gpsimd.collective_compute(
                                "AllGather", ALU.bypass, replica_groups=rg,
                                ins=[ago_in[b].opt()], outs=[ago_out[b].opt()],
                            )

                        for b in range(B):
                            for i in range(ST):
                                if b == 1 and i == 2:
                                    attn_tail(0)
                                tg = b * ST + i
                                L = (i + 1) * P
                                for hl in range(2):
                                    q_i, k_i = hl, 2 + hl
                                    Spp = ps3s.tile([P, S], F32, tag="Spp")
                                    lhsT = qkT[:, q_i, tg * P:(tg + 1) * P]
                                    for jc in range((L + 511) // 512):
                                        n0 = jc * 512
                                        n1 = min(L, n0 + 512)
                                        nc.tensor.matmul(
                                            Spp[:, n0:n1], lhsT,
                                            qkT[:, k_i,
                                                b * S + n0:b * S + n1],
                                            start=True, stop=True,
                                        )
                                    S1 = sat.tile([P, S], F32, tag="S1")
                                    nc.vector.scalar_tensor_tensor(
                                        S1[:, 0:L], Spp[:, 0:L],
                                        sc["cq"][:, tg:tg + 1],
                                        dkb[:, b * S:b * S + L],
                                        op0=ALU.mult, op1=ALU.mult,
                                    )
                                    nc.vector.tensor_tensor(
                                        S1[:, i * P:L], S1[:, i * P:L],
                                        mask[:], op=ALU.add,
                                    )
                                    negmx = sat.tile([P, 1], F32, tag="negmx")
                                    nc.vector.tensor_reduce(
                                        negmx[:], S1[:, 0:L], op=ALU.max,
                                        axis=AX.X, negate=True,
                                    )
                                    esum = sat.tile([P, 1], F32, tag="esum")
                                    nc.scalar.activation(
                                        S1[:, 0:L], S1[:, 0:L], AF.Exp,
                                        bias=negmx[:, 0:1], scale=1.0,
                                        accum_out=esum[:],
                                    )
                                    erec = sat.tile([P, 1], F32, tag="erec")
                                    nc.vector.reciprocal(erec[:], esum[:])
                                    Pb = sat.tile([P, S], BF16, tag="Pb")
                                    nc.vector.tensor_scalar(
                                        Pb[:, 0:L], S1[:, 0:L],
                                        erec[:, 0:1], None, op0=ALU.mult,
                                    )
                                    pts = sat.tile([P, ST, P], BF16,
                                                   tag="pts")
                                    nc.sync.dma_start(
                                        pts[:, 0:i + 1, :], Pb[:, 0:L],
                                        transpose=True,
                                    )
                                    att = ps3a.tile([P, P], F32, tag="att")
                                    for j in range(i + 1):
                                        nc.tensor.matmul(
                                            att[:], pts[:, j, :],
                                            v_sb[b * ST + j]
                                            [:, hl * P:(hl + 1) * P],
                                            start=(j == 0), stop=(j == i),
                                        )
                                    nc.vector.tensor_copy(
                                        att_sb[tg][:, hl * P:(hl + 1) * P],
                                        att[:],
                                    )
                                nc.vector.tensor_reduce(
                                    aso_sb[:, tg:tg + 1], att_sb[tg][:],
                                    op=ALU.max, axis=AX.X,
                                    apply_absolute_value=True,
                                )
                            # issue the per-batch scale AllReduce promptly
                            nc.sync.dma_start(
                                aso_in[b]
                                .rearrange("(t p) o -> p (t o)", p=P),
                                aso_sb[:, b * ST:(b + 1) * ST],
                            )
                            nc.gpsimd.collective_compute(
                                "AllReduce", ALU.max, replica_groups=rg,
                                ins=[aso_in[b].opt()], outs=[aso_g[b].opt()],
                            )
                        attn_tail(1)

                    # ---- Phase 4: o-proj + residual + norm2 + AG2 ----
                    with tc.tile_pool(name="pO", bufs=1) as pO, \
                         tc.tile_pool(name="psO", bufs=4, space="PSUM") as psO, \
                         tc.tile_pool(name="p4s", bufs=2) as s4, \
                         tc.tile_pool(name="pq2", bufs=2) as sq2:
                        oT = [pO.tile([P, S], BF16, tag=f"oT{k}",
                                      name=f"oT{k}", bufs=1)
                              for k in range(KT)]
                        for h in range(2):
                            for k in range(KT):
                                nc.scalar.dma_start(
                                    oT[k][:],
                                    ago_out[h][k * P:(k + 1) * P, :],
                                )
                            for tl in range(ST):
                                t = h * ST + tl
                                pso = psO.tile([P, OQ], F32, tag="pso")
                                for kb in range(KT):
                                    nc.tensor.matmul(
                                        pso[:],
                                        oT[kb][:, tl * P:(tl + 1) * P],
                                        wo_sb[kb][:],
                                        start=(kb == 0), stop=(kb == KT - 1),
                                    )
                                nc.vector.scalar_tensor_tensor(
                                    x1[t][:], pso[:],
                                    sc["sc_o"][:, t:t + 1], xc[t][:],
                                    op0=ALU.mult, op1=ALU.add,
                                )
                                nc.sync.dma_start(
                                    x1_d[t * P:(t + 1) * P, :], x1[t][:]
                                )
                                sq = s4.tile([P, OQ], F32, tag="sq4")
                                ssq_l = s4.tile([P, 1], F32, tag="ssq_l4")
                                nc.scalar.activation(
                                    sq[:], x1[t][:], AF.Square,
                                    accum_out=ssq_l[:],
                                )
                                hn = s4.tile([P, OQ], F32, tag="hn4")
                                nc.vector.tensor_tensor(
                                    hn[:], x1[t][:], n2c[:], op=ALU.mult
                                )
                                am_l = s4.tile([P, 1], F32, tag="am_l4")
                                nc.vector.tensor_reduce(
                                    am_l[:], hn[:], op=ALU.max, axis=AX.X,
                                    apply_absolute_value=True,
                                )
                                nc.sync.dma_start(
                                    ssq2_in[h][tl * P:(tl + 1) * P, :],
                                    ssq_l[:],
                                )
                                nc.sync.dma_start(
                                    am2_in[h][tl * P:(tl + 1) * P, :],
                                    am_l[:],
                                )
                            nc.gpsimd.collective_compute(
                                "AllReduce", ALU.add, replica_groups=rg,
                                ins=[ssq2_in[h].opt()],
                                outs=[ssq2_g[h].opt()],
                            )
                            nc.gpsimd.collective_compute(
                                "AllReduce", ALU.max, replica_groups=rg,
                                ins=[am2_in[h].opt()],
                                outs=[am2_g[h].opt()],
                            )
                            cols = slice(h * ST, (h + 1) * ST)
                            rv = sc["tmp1"]
                            nc.sync.dma_start(
                                rv[:, cols],
                                ssq2_g[h]
                                .rearrange("(t p) o -> p (t o)", p=P),
                            )
                            nc.vector.tensor_scalar(
                                rv[:, cols], rv[:, cols], 1.0 / D, 1e-6,
                                op0=ALU.mult, op1=ALU.add,
                            )
                            nc.scalar.activation(
                                rv[:, cols], rv[:, cols], AF.Sqrt
                            )
                            nc.vector.reciprocal(rv[:, cols], rv[:, cols])
                            av = sc["tmp2"]
                            nc.sync.dma_start(
                                av[:, cols],
                                am2_g[h]
                                .rearrange("(t p) o -> p (t o)", p=P),
                            )
                            nc.vector.tensor_tensor(
                                av[:, cols], av[:, cols], rv[:, cols],
                                op=ALU.mult,
                            )
                            nc.vector.tensor_scalar(
                                av[:, cols], av[:, cols], 1e-8, None,
                                op0=ALU.add,
                            )
                            nc.vector.reciprocal(
                                sc["f2"][:, cols], av[:, cols]
                            )
                            nc.vector.tensor_tensor(
                                sc["f2"][:, cols], sc["f2"][:, cols],
                                rv[:, cols], op=ALU.mult,
                            )
                            nc.vector.tensor_scalar(
                                sc["f2"][:, cols], sc["f2"][:, cols],
                                127.0, None, op0=ALU.mult,
                            )
                            nc.vector.tensor_scalar(
                                sc["sc_g"][:, cols], av[:, cols],
                                cb[:, 3:4], None, op0=ALU.mult,
                            )
                            nc.vector.tensor_scalar(
                                sc["sc_u"][:, cols], av[:, cols],
                                cb[:, 4:5], None, op0=ALU.mult,
                            )
                            for tl in range(ST):
                                t = h * ST + tl
                                tmp = sq2.tile([P, OQ], F32, tag="q2t")
                                nc.vector.scalar_tensor_tensor(
                                    tmp[:], x1[t][:], sc["f2"][:, t:t + 1],
                                    n2c[:], op0=ALU.mult, op1=ALU.mult,
                                )
                                nc.vector.tensor_scalar(
                                    tmp[:], tmp[:], MAGIC, None, op0=ALU.add
                                )
                                a_q2 = sq2.tile([P, OQ], BF16, tag="a_q2")
                                nc.vector.tensor_scalar(
                                    a_q2[:], tmp[:], -MAGIC, None,
                                    op0=ALU.add,
                                )
                                tq2 = sq2.tile([P, 2, P], BF16, tag="tq2")
                                nc.sync.dma_start(
                                    tq2[:], a_q2[:], transpose=True
                                )
                                nc.sync.dma_start(
                                    ag2_in[h]
                                    .rearrange("(kb p) r -> p kb r", p=P)
                                    [:, :, tl * P:(tl + 1) * P],
                                    tq2[:],
                                )
                            nc.gpsimd.collective_compute(
                                "AllGather", ALU.bypass, replica_groups=rg,
                                ins=[ag2_in[h].opt()],
                                outs=[ag2_out[h].opt()],
                            )
                # pL2, pL1 close here

            # ---- Phase 5 + 6: MLP ----
            with tc.tile_pool(name="pDw", bufs=1) as pDw:
                wd_sb = [pDw.tile([P, OQ], BF16, tag=f"wd{k}", name=f"wd{k}")
                         for k in range(MLP // P)]
                with tc.tile_pool(name="pM", bufs=1) as pM:
                    a2t = [pM.tile([P, S], BF16, tag=f"a2t{k}",
                                   name=f"a2t{k}", bufs=1)
                           for k in range(KT)]
                    asm_sb = pM.tile([P, RT], F32, tag="asm_sb")
                    with tc.tile_pool(name="p5s", bufs=2) as s5, \
                         tc.tile_pool(name="ps5", bufs=2,
                                      space="PSUM") as ps5:
                        for h in range(2):
                            for k in range(KT):
                                nc.scalar.dma_start(
                                    a2t[k][:],
                                    ag2_out[h][k * P:(k + 1) * P, :],
                                )
                            if h == 1:
                                # wd loads run here, mid-MLP, in reserved
                                # space on an otherwise busy-free window
                                for k in range(MLP // P):
                                    nc.scalar.dma_start(
                                        wd_sb[k][:],
                                        wd_b[k * P:(k + 1) * P, :],
                                    )
                            for gl in range(2):
                                g = h * 2 + gl
                                m_tiles = {}
                                for tl4 in range(4):
                                    tl = gl * 4 + tl4
                                    t = h * ST + tl
                                    psg = ps5.tile([P, OM], F32, tag="psg")
                                    psu = ps5.tile([P, OM], F32, tag="psu")
                                    for kb in range(KT):
                                        lhsT = a2t[kb][
                                            :, tl * P:(tl + 1) * P]
                                        for n in range(2):
                                            nc.tensor.matmul(
                                                psg[:, n * 512:
                                                    (n + 1) * 512],
                                                lhsT,
                                                wgu_sb[kb][
                                                    :, n * 512:
                                                    (n + 1) * 512],
                                                start=(kb == 0),
                                                stop=(kb == KT - 1),
                                            )
                                    for kb in range(KT):
                                        lhsT = a2t[kb][
                                            :, tl * P:(tl + 1) * P]
                                        for n in range(2):
                                            nc.tensor.matmul(
                                                psu[:, n * 512:
                                                    (n + 1) * 512],
                                                lhsT,
                                                wgu_sb[kb][
                                                    :, OM + n * 512:
                                                    OM + (n + 1) * 512],
                                                start=(kb == 0),
                                                stop=(kb == KT - 1),
                                            )
                                    sig = s5.tile([P, OM], F32, tag="sig")
                                    nc.scalar.activation(
                                        sig[:], psg[:], AF.Sigmoid,
                                        scale=sc["sc_g"][:, t:t + 1],
                                    )
                                    sgl = s5.tile([P, OM], F32, tag="sgl")
                                    nc.vector.scalar_tensor_tensor(
                                        sgl[:], psg[:],
                                        sc["sc_g"][:, t:t + 1], sig[:],
                                        op0=ALU.mult, op1=ALU.mult,
                                    )
                                    mt = s5.tile([P, OM], F32,
                                                 tag=f"m{tl4}",
                                                 name=f"m{tl4}", bufs=2)
                                    m_tiles[tl4] = mt
                                    nc.vector.scalar_tensor_tensor(
                                        mt[:], psu[:],
                                        sc["sc_u"][:, t:t + 1], sgl[:],
                                        op0=ALU.mult, op1=ALU.mult,
                                    )
                                    nc.vector.tensor_reduce(
                                        asm_sb[:, t:t + 1], mt[:],
                                        op=ALU.max, axis=AX.X,
                                        apply_absolute_value=True,
                                    )
                                nc.sync.dma_start(
                                    asm_in[g]
                                    .rearrange("(t p) o -> p (t o)", p=P),
                                    asm_sb[:, g * 4:(g + 1) * 4],
                                )
                                nc.gpsimd.collective_compute(
                                    "AllReduce", ALU.max, replica_groups=rg,
                                    ins=[asm_in[g].opt()],
                                    outs=[asm_g[g].opt()],
                                )
                                gs = slice(g * 4, (g + 1) * 4)
                                nc.sync.dma_start(
                                    sc["asmg"][:, gs],
                                    asm_g[g]
                                    .rearrange("(t p) o -> p (t o)", p=P),
                                )
                                nc.vector.tensor_scalar(
                                    sc["asmg"][:, gs], sc["asmg"][:, gs],
                                    1e-8, None, op0=ALU.add,
                                )
                                nc.vector.reciprocal(
                                    sc["qsm"][:, gs], sc["asmg"][:, gs]
                                )
                                nc.vector.tensor_scalar(
                                    sc["qsm"][:, gs], sc["qsm"][:, gs],
                                    127.0, None, op0=ALU.mult,
                                )
                                nc.vector.tensor_scalar(
                                    sc["sc_d"][:, gs], sc["asmg"][:, gs],
                                    cb[:, 5:6], None, op0=ALU.mult,
                                )
                                for tl4 in range(4):
                                    tl = gl * 4 + tl4
                                    t = h * ST + tl
                                    tmp = s5.tile([P, OM], F32, tag="qm")
                                    nc.vector.tensor_scalar(
                                        tmp[:], m_tiles[tl4][:],
                                        sc["qsm"][:, t:t + 1], MAGIC,
                                        op0=ALU.mult, op1=ALU.add,
                                    )
                                    m_q = s5.tile([P, OM], BF16, tag="m_q")
                                    nc.vector.tensor_scalar(
                                        m_q[:], tmp[:], -MAGIC, None,
                                        op0=ALU.add,
                                    )
                                    tqm = s5.tile([P, KM, P], BF16,
                                                  tag="tqm")
                                    nc.sync.dma_start(
                                        tqm[:], m_q[:], transpose=True
                                    )
                                    nc.sync.dma_start(
                                        agm_in[h]
                                        .rearrange("(kb p) r -> p kb r",
                                                   p=P)
                                        [:, :, tl * P:(tl + 1) * P],
                                        tqm[:],
                                    )
                            nc.gpsimd.collective_compute(
                                "AllGather", ALU.bypass, replica_groups=rg,
                                ins=[agm_in[h].opt()],
                                outs=[agm_out[h].opt()],
                            )

                # ---- Phase 6: down ----
                with tc.tile_pool(name="p6s", bufs=3) as s6, \
                     tc.tile_pool(name="ps6", bufs=1, space="PSUM") as ps6:
                    for hh in range(2):
                        psd = [ps6.tile([P, OQ], F32, tag=f"psd{tl}",
                                        name=f"psd{tl}")
                               for tl in range(ST)]
                        for kb in range(MLP // P):
                            mt_l = s6.tile([P, S], BF16, tag="mt_l")
                            nc.scalar.dma_start(
                                mt_l[:],
                                agm_out[hh][kb * P:(kb + 1) * P, :],
                            )
                            for tl in range(ST):
                                nc.tensor.matmul(
                                    psd[tl][:],
                                    mt_l[:, tl * P:(tl + 1) * P],
                                    wd_sb[kb][:],
                                    start=(kb == 0),
                                    stop=(kb == MLP // P - 1),
                                )
                        for tl in range(ST):
                            t = hh * ST + tl
                            x1r = s6.tile([P, OQ], F32, tag="x1r")
                            nc.scalar.dma_start(
                                x1r[:], x1_d[t * P:(t + 1) * P, :]
                            )
                            ot2 = s6.tile([P, OQ], F32, tag="ot2")
                            nc.vector.scalar_tensor_tensor(
                                ot2[:], psd[tl][:], sc["sc_d"][:, t:t + 1],
                                x1r[:], op0=ALU.mult, op1=ALU.add,
                            )
                            nc.sync.dma_start(
                                out_d[t * P:(t + 1) * P, :], ot2[:]
                            )

    nc.compile()
    return nc


def _prep_in_maps(inputs):
    x = np.asarray(inputs["x"], np.float32).reshape(R, D)
    wq = np.asarray(inputs["wq"], np.float32)
    wk = np.asarray(inputs["wk"], np.float32)
    wv = np.asarray(inputs["wv"], np.float32)
    wo = np.asarray(inputs["wo"], np.float32)
    wg = np.asarray(inputs["wg"], np.float32)
    wu = np.asarray(inputs["wu"], np.float32)
    wd = np.asarray(inputs["wd"], np.float32)
    n1 = np.asarray(inputs["norm1_w"], np.float32).reshape(1, D)
    n2 = np.asarray(inputs["norm2_w"], np.float32).reshape(1, D)

    def wscale(w):
        return float(np.abs(w.astype(np.float64)).mean()) + 1e-8

    def tern(w, ws):
        return np.clip(np.round(w / np.float32(ws)), -1.0, 1.0) \
            .astype(ml_dtypes.bfloat16)

    ws_q, ws_k, ws_v = wscale(wq), wscale(wk), wscale(wv)
    ws_o, ws_g, ws_u, ws_d = wscale(wo), wscale(wg), wscale(wu), wscale(wd)
    wq_t = tern(wq, ws_q)
    wk_t = tern(wk, ws_k)
    wv_t = tern(wv, ws_v)
    wo_t = tern(wo, ws_o)
    wg_t = tern(wg, ws_g)
    wu_t = tern(wu, ws_u)
    wd_t = tern(wd, ws_d)

    csc = np.array([[
        ws_q * ws_k * INV_SQRT_HD / (127.0 * 127.0),
        ws_v / 127.0, ws_o / 127.0, ws_g / 127.0, ws_u / 127.0,
        ws_d / 127.0, 0.0, 0.0,
    ]], np.float32)
    iv, jv = np.mgrid[0:P, 0:P]
    causal = np.where(jv <= iv, 0.0, -1e30).astype(np.float32)

    in_maps = []
    for c in range(NCORES):
        qs = slice(c * OQ, (c + 1) * OQ)
        ms = slice(c * OM, (c + 1) * OM)
        in_maps.append({
            "x_rows": np.ascontiguousarray(x[c * RL:(c + 1) * RL]),
            "x_cols": np.ascontiguousarray(x[:, qs]),
            "wqkv_b": np.ascontiguousarray(
                np.concatenate([wq_t[qs], wk_t[qs], wv_t[qs]], 0).T
            ),
            "wo_b": np.ascontiguousarray(wo_t[qs].T),
            "wgu_b": np.ascontiguousarray(
                np.concatenate([wg_t[ms], wu_t[ms]], 0).T
            ),
            "wd_b": np.ascontiguousarray(wd_t[qs].T),
            "norm1_w": n1,
            "n2c_w": np.ascontiguousarray(n2[:, qs]),
            "csc": csc,
            "causal": causal,
        })
    return in_maps


def _assemble(results) -> np.ndarray:
    out = np.empty((R, D), np.float32)
    for c in range(NCORES):
        out[:, c * OQ:(c + 1) * OQ] = results[c]["out"]
    return out.reshape(B, S, D)


def kernel(**inputs) -> np.ndarray:
    global _CACHED_NC
    if _CACHED_NC is None:
        _CACHED_NC = build_program()
    nc = _CACHED_NC
    in_maps = _prep_in_maps(inputs)
    res = run_bass_kernel_spmd(nc, in_maps, core_ids=list(range(NCORES)))
    return _assemble(res.results).astype(np.float32)
